# revision 3
# baseline (speedup 1.0000x reference)
"""Trainium2 Bass kernel v2 for nn_CNNConcatLinear (B=1024, N=24, PD=2, C=512).

Rank-3 restructure: h = csl1(x) has rank-3 structure over the sequence dim
(h[b] = Xa[b] (N x 3) @ Wb[b] (3 x F)), so conv(h) = A[b] @ Toeplitz(x[b])
where A[b][(c,r,d),co] = sum_q v_r[b,q] * convW[co,q,d] comes from a dense
GEMM (8x fewer MACs than direct convs). Post-gate hyper-biases h3/h4 are
absorbed into later layers via host-folded nctx blocks (h4c = c4w@c3_hw,
hlc = clw@c4_hw), leaving one DVE op per CSL epilogue.
"""

import math
import os

import numpy as np

B, N, PD, C = 1024, 24, 2, 512
F = 2 * C
NCORES = 8
BLOC = B // NCORES          # 128 batch per core
BC = 16                     # batch chunk for c3/c4/cl
NBC = BLOC // BC            # 8
PADL = 5
NW = N + 2 * PADL           # 34

# per-chunk tap structure (d ascending). chunk c covers co[c*128:(c+1)*128]
MAXD = [0, 0, 0, 0, 1, 1, 3, 5]
TAPS_C = {c: list(range(-MAXD[c], MAXD[c] + 1)) for c in range(8)}
KD_C = [len(TAPS_C[c]) for c in range(8)]          # 1,1,1,1,3,3,7,11
KC_C = [3 * k for k in KD_C]                        # 3,3,3,3,9,9,21,33
# class-major, d-major-within-class block order (enables merged scatters)
_CLS_CHUNKS = [[0, 1, 2, 3], [4, 5], [6], [7]]
BLK = {}
for _chs in _CLS_CHUNKS:
    for _d in TAPS_C[_chs[0]]:
        for _c in _chs:
            BLK[(_c, _d)] = len(BLK)
NBLK = len(BLK)             # 28

# apply classes: (chunks, Kd, pack, groups)
CLASSES = [
    ([0, 1, 2, 3], 1, 16, 8),
    ([4, 5], 3, 8, 16),
    ([6], 7, 4, 32),
    ([7], 11, 2, 64),
]
CHUNK_CLS = {}
for _ci, (_chs, _kd, _pk, _gr) in enumerate(CLASSES):
    for _c in _chs:
        CHUNK_CLS[_c] = _ci

LAST_RESULTS = None


def _pe_table():
    pos = np.arange(N, dtype=np.float32)[:, None]
    div = np.exp(np.arange(0, F, 2, dtype=np.float32) * (-np.log(10000.0) / F))
    pe = np.zeros((N, F), dtype=np.float32)
    pe[:, 0::2] = np.sin(pos * div)
    pe[:, 1::2] = np.cos(pos * div)
    return pe


def _f32(a):
    return np.ascontiguousarray(np.asarray(a, dtype=np.float32))


def _bf16(a):
    import ml_dtypes
    return np.ascontiguousarray(
        np.asarray(a, dtype=np.float32).astype(ml_dtypes.bfloat16)).view(np.uint8)


def _build(host, num_devices=NCORES):
    import concourse.bass as bass
    import concourse.mybir as mybir
    import concourse.tile as tile
    from concourse.bass_types import AP
    from concourse import bacc
    from concourse.masks import make_identity

    f32 = mybir.dt.float32
    bf16 = mybir.dt.bfloat16
    AluOp = mybir.AluOpType
    Act = mybir.ActivationFunctionType

    M3, v3, s3 = host["M3"], host["v3"], host["s3"]
    DEBUG = bool(int(os.environ.get("KERNEL_DEBUG", "0")))

    nc = bacc.Bacc("TRN2", target_bir_lowering=False, debug=False,
                   num_devices=num_devices)

    def din(name, shape, dt=bf16):
        return nc.dram_tensor(name, list(shape), dt, kind="ExternalInput").ap()

    ctx_d = din("ctx", [BLOC, C], f32)
    beta_d = din("betav", [BLOC, 1], f32)
    wg_d = din("wg", [128, 25 * 4, 128])
    gbias8_d = din("gbias8", [1, 25 * 128])
    ones1_d = din("ones1", [1, 128])
    convt_d = din("convt", [8, 128, NBLK * 128])
    c1wb_d = din("c1wb", [128, 8, 3], f32)
    c3wt_d = din("c3wt", [128, 8, C])
    c3bT_d = din("c3bT", [N, C])
    sel24_d = din("sel24", [N, BC * N])
    selbe_d = din("selbe", [64, BC * N])
    selbo_d = din("selbo", [64, BC * N])
    c4wt_d = din("c4wt", [128, 4, 256])
    c4b_d = din("c4b", [2, 128], f32)
    clwt_d = din("clwt", [128, 2, PD])
    clb_d = din("clb", [PD, 1], f32)
    idb_d = din("idb", [128, 128])
    tpl_d = [din(f"tpl{ci}", [KC_C[cls[0][0]] * cls[2], 128 * N])
             for ci, cls in enumerate(CLASSES)]
    af_d = nc.dram_tensor("afscr", [7, BLOC, 3, 512], bf16,
                          kind="Internal").ap()
    out_d = nc.dram_tensor("out", [BLOC * N, PD], f32, kind="ExternalOutput").ap()
    if DEBUG:
        dbg_af = nc.dram_tensor("dbg_af", [7, BLOC, 3, 512], bf16,
                                kind="ExternalOutput").ap()
        dbg_y = nc.dram_tensor("dbg_y", [8, 128, BLOC, N], bf16,
                               kind="ExternalOutput").ap()
        dbg_t3 = nc.dram_tensor("dbg_t3", [4, 128, BC, N], bf16,
                                kind="ExternalOutput").ap()
        dbg_t4 = nc.dram_tensor("dbg_t4", [2, 128, BC, N], bf16,
                                kind="ExternalOutput").ap()
        dbg_of = nc.dram_tensor("dbg_of", [PD, BC, N], f32,
                                kind="ExternalOutput").ap()
        dbg_gl = nc.dram_tensor("dbg_gl", [PD, 128], f32,
                                kind="ExternalOutput").ap()
        dbg_hl = nc.dram_tensor("dbg_hl", [PD, 128], f32,
                                kind="ExternalOutput").ap()
        dbg_psl = nc.dram_tensor("dbg_psl", [PD, BC, N], f32,
                                 kind="ExternalOutput").ap()

    with tile.TileContext(nc) as tc:
        import contextlib
        est = contextlib.ExitStack()
        with est:
            wp = est.enter_context(tc.tile_pool(name="wp", bufs=1))

            # ---------------- persistent tiles + early DMAs ----------------
            beta_t = wp.tile([128, 1], f32, tag="beta")
            nc.sync.dma_start(beta_t[:], beta_d[:])
            ctx_t = wp.tile([128, C], f32, tag="ctx")
            nc.sync.dma_start(ctx_t[:], ctx_d[:])
            wgp_cm = tc.tile_pool(name="wgp", bufs=1)
            wgp = wgp_cm.__enter__()
            wg_ts = []
            for w5 in range(9):
                lo, hi = w5 * 12, min(100, (w5 + 1) * 12)
                wt = wgp.tile([128, 12, 128], bf16, tag=f"wgs{w5 % 2}",
                              name=f"wgs{w5}")
                eng = nc.sync if w5 % 2 == 0 else nc.gpsimd
                eng.dma_start(wt[:, 0:hi - lo, :], wg_d[:, lo:hi, :])
                wg_ts.append(wt)
            gbias8_s = wp.tile([1, 25, 128], bf16, tag="gbias8")
            nc.sync.dma_start(gbias8_s[:], gbias8_d.rearrange("o (c p) -> o c p", c=25, p=128))
            ones1_s = wp.tile([1, 128], bf16, tag="ones1")
            nc.sync.dma_start(ones1_s[:], ones1_d[:])
            c1wb_s = wp.tile([128, 8, 3], f32, tag="c1wb")
            nc.sync.dma_start(c1wb_s[:], c1wb_d[:])
            c3wt_s = wp.tile([128, 8, C], bf16, tag="c3wt")
            nc.sync.dma_start(c3wt_s[:], c3wt_d[:])
            c3bT_s = wp.tile([N, C], bf16, tag="c3bT")
            nc.sync.dma_start(c3bT_s[:], c3bT_d[:])
            sel24_s = wp.tile([N, BC * N], bf16, tag="sel24")
            nc.sync.dma_start(sel24_s[:], sel24_d[:])
            selbe_s = wp.tile([64, BC * N], bf16, tag="selbe")
            nc.sync.dma_start(selbe_s[:], selbe_d[:])
            selbo_s = wp.tile([64, BC * N], bf16, tag="selbo")
            nc.sync.dma_start(selbo_s[:], selbo_d[:])
            c4wt_s = wp.tile([128, 4, 256], bf16, tag="c4wt")
            nc.sync.dma_start(c4wt_s[:], c4wt_d[:])
            c4b_s = wp.tile([128, 2], f32, tag="c4b")
            nc.sync.dma_start(c4b_s[:], c4b_d.rearrange("m p -> p m"))
            clwt_s = wp.tile([128, 2, PD], bf16, tag="clwt")
            nc.sync.dma_start(clwt_s[:], clwt_d[:])
            clb_s = wp.tile([PD, 1], f32, tag="clb")
            nc.sync.dma_start(clb_s[:], clb_d[:])
            idb_s = wp.tile([128, 128], bf16, tag="idb")
            nc.gpsimd.dma_start(idb_s[:], idb_d[:])
            ident = wp.tile([128, 128], f32, tag="ident")
            make_identity(nc, ident[:])

            # Toeplitz tiles (host-built, one DMA per class)
            T_s = []
            for ci, (chs, kd, pack, groups) in enumerate(CLASSES):
                kc = 3 * kd
                t = wp.tile([kc * pack, 128 * N], bf16, tag=f"T{ci}", name=f"T{ci}")
                nc.gpsimd.dma_start(t[:], tpl_d[ci][:])
                T_s.append(t)


            convt_s = []
            for k in range(8):
                t = wp.tile([128, NBLK * 128], bf16, tag=f"convt{k}",
                            name=f"convt{k}")
                eng = nc.sync if k % 2 == 0 else nc.gpsimd
                eng.dma_start(t[:], convt_d[k])
                convt_s.append(t)

            # L tiles (scatter destinations): one per class,
            # free layout [i-group, c-in-class, co]
            L_s = []
            for ci, (chs, kd, pack, groups) in enumerate(CLASSES):
                L_s.append(wp.tile([3 * kd * pack, groups * len(chs) * 128],
                                   bf16, tag=f"L{ci}", name=f"Lc{ci}"))

            # phase-B output tiles
            g3_s = wp.tile([128, 4, 128], f32, tag="g3")
            g4_s = wp.tile([128, 2, 128], f32, tag="g4")

            h4cT = [wp.tile([64, 2, 128], bf16, tag=f"h4cT{h}", name=f"h4cT{h}")
                    for h in range(2)]
            gl_s = wp.tile([PD, 128], f32, tag="gl")
            hl_s = wp.tile([PD, 128], f32, tag="hl")

            hlcT = [wp.tile([64, PD], bf16, tag=f"hlcT{h}", name=f"hlcT{h}")
                    for h in range(2)]
            v_s = wp.tile([128, 8, 3, 128], bf16, tag="v")
            nctxT = wp.tile([128, C], bf16, tag="nctxT")

            # ---------------- phase A: new_ctx ----------------
            with tc.tile_pool(name="pa", bufs=1) as pap, \
                 tc.tile_pool(name="ps_a", bufs=1, space="PSUM") as psa:
                sinb = pap.tile([128, 1], f32, tag="sinb")
                nc.scalar.activation(sinb[:], beta_t[:], Act.Sin)
                cosb = pap.tile([128, 1], f32, tag="cosb")
                nc.vector.tensor_scalar_add(cosb[:], beta_t[:], math.pi / 2)
                nc.scalar.activation(cosb[:], cosb[:], Act.Sin)

                u = pap.tile([128, 3], f32, tag="u")
                for j in range(3):
                    uj = u[:, j:j + 1]
                    nc.vector.tensor_scalar(uj, beta_t[:], float(M3[j, 0]),
                                            float(v3[j]), AluOp.mult, AluOp.add)
                    nc.vector.scalar_tensor_tensor(uj, sinb[:], float(M3[j, 1]),
                                                   uj, AluOp.mult, AluOp.add)
                    nc.vector.scalar_tensor_tensor(uj, cosb[:], float(M3[j, 2]),
                                                   uj, AluOp.mult, AluOp.add)

                ej = None
                z = pap.tile([128, C], f32, tag="z")
                num = pap.tile([128, C], f32, tag="num")
                tvec = [beta_t, sinb, cosb]
                for j in range(3):
                    ej = psa.tile([128, C], f32, tag="ej", bufs=3)
                    nc.scalar.activation(ej[:], ctx_t[:], Act.Exp,
                                         bias=u[:, j:j + 1], scale=float(s3[j]))
                    if j == 0:
                        nc.vector.tensor_copy(z[:], ej[:])
                        nc.vector.tensor_scalar(num[:], ej[:], tvec[j][:], None,
                                                AluOp.mult)
                    else:
                        nc.vector.tensor_add(z[:], z[:], ej[:])
                        nc.vector.scalar_tensor_tensor(num[:], ej[:], tvec[j][:],
                                                       num[:], AluOp.mult,
                                                       AluOp.add)
                nc.scalar.activation(z[:], z[:], Act.Ln)
                nc.scalar.activation(z[:], z[:], Act.Exp, scale=-1.0)
                nc.vector.tensor_mul(num[:], num[:], z[:])
                nctx = ctx_t
                nc.vector.tensor_add(nctx[:], ctx_t[:], num[:])

                for kb in range(4):
                    pst = psa.tile([128, 128], f32, tag="tr")
                    nc.tensor.transpose(pst[:],
                                        nctx[:, kb * 128:(kb + 1) * 128],
                                        ident[:])
                    nc.scalar.copy(nctxT[:, kb * 128:(kb + 1) * 128], pst[:])

            # ---------------- phase B: gates (25 blocks, bf16) ----------------
            # blocks: 0-7 g1, 8-15 h1, 16-19 g3, 20-21 g4, 22-23 h4c, 24 misc
            gw_cm = tc.tile_pool(name="gw", bufs=1)
            gwp = gw_cm.__enter__()
            pb_cm = tc.tile_pool(name="ps_b", bufs=4, space="PSUM")
            pbp = pb_cm.__enter__()
            if True:
                g1_s = gwp.tile([128, 8, 128], bf16, tag="g1")
                h1_s = gwp.tile([128, 8, 128], bf16, tag="h1")
                h4c_sb = gwp.tile([128, 2, 128], bf16, tag="h4csb")
                hlc_sb = gwp.tile([PD, 128], bf16, tag="hlcsb")
                gp_bank = None
                for c in range(16):
                    if c % 4 == 0:
                        gp_bank = pbp.tile([128, 4, 128], f32, tag="gps")
                    if False:
                        for k in range(4):
                            rhs = nctxT[:, k * 128:(k + 1) * 128]
                            sidx = c * 4 + k
                            wslice = wg_ts[sidx // 12][:, sidx % 12, :]
                            for q3 in range(3):
                                nc.tensor.matmul(
                                    gp_bank[0:2, q3, :],
                                    wslice[:, q3 * 2:q3 * 2 + 2], rhs,
                                    start=(k == 0 and q3 == 0), stop=False)
                        for q3 in range(3):
                            nc.tensor.matmul(
                                gp_bank[0:2, q3, :],
                                gbias8_s[:, 24, q3 * 2:q3 * 2 + 2],
                                ones1_s[:], start=False, stop=(q3 == 2))
                        nc.scalar.activation(gl_s[:], gp_bank[0:2, 0, :],
                                             Act.Sigmoid)
                        nc.scalar.copy(hl_s[:], gp_bank[0:2, 1, :])
                        nc.scalar.copy(hlc_sb[:], gp_bank[0:2, 2, :])
                        if DEBUG:
                            nc.scalar.dma_start(dbg_gl[:], gl_s[:])
                            nc.scalar.dma_start(dbg_hl[:], hl_s[:])
                        continue
                    for k in range(4):
                        rhs = nctxT[:, k * 128:(k + 1) * 128]
                        sidx = c * 4 + k
                        wslice = wg_ts[sidx // 12][:, sidx % 12, :]
                        nc.tensor.matmul(gp_bank[:, c % 4, :],
                                         wslice, rhs,
                                         start=(k == 0), stop=False)
                    nc.tensor.matmul(gp_bank[:, c % 4, :],
                                     gbias8_s[:, c, :], ones1_s[:],
                                     start=False, stop=True)
                    if c % 2 == 0:
                        continue
                    ps2 = gp_bank[:, (c % 4) - 1:(c % 4) + 1, :]
                    if c < 8:
                        nc.scalar.activation(g1_s[:, c - 1:c + 1, :], ps2,
                                             Act.Sigmoid)
                    else:
                        nc.scalar.copy(h1_s[:, c - 9:c - 7, :], ps2)

                # transposes for SELB-matmul lhsTs
                # v-fold: v_r[q, b] = g1*c1w_r (+h1 for r=2)
                for k in range(8):
                    for r in range(3):
                        if r < 2:
                            nc.vector.tensor_scalar(
                                v_s[:, k, r, :], g1_s[:, k, :],
                                c1wb_s[:, k, r:r + 1], None, AluOp.mult)
                        else:
                            nc.vector.scalar_tensor_tensor(
                                v_s[:, k, r, :], g1_s[:, k, :],
                                c1wb_s[:, k, 2:3], h1_s[:, k, :],
                                AluOp.mult, AluOp.add)

                def _phase_b2():
                    gp_bank = None
                    for c in range(16, 25):
                        if c % 4 == 0:
                            gp_bank = pbp.tile([128, 4, 128], f32, tag="gps",
                                               bufs=4)
                        if c == 24:
                            for k in range(4):
                                rhs = nctxT[:, k * 128:(k + 1) * 128]
                                sidx = c * 4 + k
                                wslice = wg_ts[sidx // 12][:, sidx % 12, :]
                                for q3 in range(3):
                                    nc.tensor.matmul(
                                        gp_bank[0:2, q3, :],
                                        wslice[:, q3 * 2:q3 * 2 + 2], rhs,
                                        start=(k == 0 and q3 == 0), stop=False)
                            for q3 in range(3):
                                nc.tensor.matmul(
                                    gp_bank[0:2, q3, :],
                                    gbias8_s[:, 24, q3 * 2:q3 * 2 + 2],
                                    ones1_s[:], start=False, stop=(q3 == 2))
                            nc.scalar.activation(gl_s[:], gp_bank[0:2, 0, :],
                                                 Act.Sigmoid)
                            nc.scalar.copy(hl_s[:], gp_bank[0:2, 1, :])
                            nc.scalar.copy(hlc_sb[:], gp_bank[0:2, 2, :])
                            if DEBUG:
                                nc.scalar.dma_start(dbg_gl[:], gl_s[:])
                                nc.scalar.dma_start(dbg_hl[:], hl_s[:])
                            continue
                        for k in range(4):
                            rhs = nctxT[:, k * 128:(k + 1) * 128]
                            sidx = c * 4 + k
                            wslice = wg_ts[sidx // 12][:, sidx % 12, :]
                            nc.tensor.matmul(gp_bank[:, c % 4, :],
                                             wslice, rhs,
                                             start=(k == 0), stop=False)
                        nc.tensor.matmul(gp_bank[:, c % 4, :],
                                         gbias8_s[:, c, :], ones1_s[:],
                                         start=False, stop=True)
                        if c % 2 == 0:
                            continue
                        ps2 = gp_bank[:, (c % 4) - 1:(c % 4) + 1, :]
                        if c < 20:
                            nc.scalar.activation(g3_s[:, c - 17:c - 15, :], ps2,
                                                 Act.Sigmoid)
                        elif c < 22:
                            nc.scalar.activation(g4_s[:, 0:2, :], ps2,
                                                 Act.Sigmoid)
                        else:
                            nc.scalar.copy(h4c_sb[:, 0:2, :], ps2)

                    for h in range(2):
                        for m2 in range(2):
                            pst = pbp.tile([128, 4, 128], f32, tag="gps",
                                           bufs=4)
                            pb = pst[0:64, 0, :].bitcast(bf16)[:, 0:128]
                            nc.tensor.transpose(
                                pb, h4c_sb[:, m2, h * 64:(h + 1) * 64],
                                idb_s[:])
                            nc.scalar.copy(h4cT[h][:, m2, :], pb)
                        pst = pbp.tile([128, 4, 128], f32, tag="gps", bufs=4)
                        pb = pst[0:64, 0, :].bitcast(bf16)[:, 0:PD]
                        nc.tensor.transpose(pb, hlc_sb[:, h * 64:(h + 1) * 64],
                                            idb_s[0:PD, 0:PD])
                        nc.scalar.copy(hlcT[h][:], pb)

            # ---------------- fold GEMM (bank-major) + afdram + scatters ----
            # scatter units per bank: (ci, r, d, first_blk, n_chunks)
            bank_units = {b: [] for b in range(7)}
            for ci, (chs, kd, pack, groups) in enumerate(CLASSES):
                maxd = (kd - 1) // 2
                for didx in range(kd):
                    d = didx - maxd
                    fb = BLK[(chs[0], d)]
                    assert all(BLK[(c, d)] == fb + ii
                               for ii, c in enumerate(chs))
                    for r in range(3):
                        bank_units[fb // 4].append(
                            (ci, r, didx, fb, len(chs)))
            scat_engs = [nc.sync, nc.gpsimd, nc.sync, nc.scalar]
            ns = 0
            with tc.tile_pool(name="afst", bufs=3) as afp, \
                 tc.tile_pool(name="ps_f", bufs=3, space="PSUM") as pfp:
                for bank in range(7):
                    stage = afp.tile([128, 3, 512], bf16, tag="afstg")
                    for r in range(3):
                        pr = pfp.tile([128, 512], f32, tag="pr")
                        for k in range(8):
                            nc.tensor.matmul(
                                pr[:], v_s[:, k, r, :],
                                convt_s[k][:, bank * 512:(bank + 1) * 512],
                                start=(k == 0), stop=(k == 7))
                        nc.scalar.copy(stage[:, r, :], pr[:])
                    nc.sync.dma_start(af_d[bank], stage[:])
                    if DEBUG:
                        nc.scalar.dma_start(dbg_af[bank], stage[:])

                    for (ci, r, didx, fb, nch) in bank_units[bank]:
                        chs, kd, pack, groups = CLASSES[ci]
                        kc = 3 * kd
                        lfree = groups * len(chs) * 128
                        t = r * kd + didx
                        span = nch * 128
                        dst = AP(L_s[ci][:].tensor, t * lfree,
                                 [[kc * lfree, pack], [span, groups],
                                  [1, span]])
                        src = AP(af_d.tensor,
                                 bank * BLOC * 3 * 512 + r * 512
                                 + (fb % 4) * 128,
                                 [[3 * 512, pack],
                                  [pack * 3 * 512, groups], [1, span]])
                        scat_engs[ns % 4].dma_start(dst, src)
                        ns += 1


            _phase_b2()
            pb_cm.__exit__(None, None, None)
            gw_cm.__exit__(None, None, None)
            wgp_cm.__exit__(None, None, None)

            # ---------------- phase C: c3/c4/cl per batch chunk ----------------
            def bcast(ap_2d, np_=N):
                return ap_2d.unsqueeze(2).broadcast_to(
                    [ap_2d.shape[0], BC, np_])

            def cpy(idx, dst, src):
                if idx % 2 == 0:
                    nc.scalar.copy(dst, src)
                else:
                    nc.vector.tensor_copy(dst, src)
            with tc.tile_pool(name="yp", bufs=2) as ypool, \
                 tc.tile_pool(name="ps_y", bufs=3, space="PSUM") as pyp, \
                 tc.tile_pool(name="t3p", bufs=4) as t3p, \
                 tc.tile_pool(name="obp", bufs=3) as obp, \
                 tc.tile_pool(name="ps_c3", bufs=2, space="PSUM") as ps_c3, \
                 tc.tile_pool(name="ps_ms", bufs=2, space="PSUM") as ps_ms:
                for bc in range(NBC):
                    cs = bc * BC
                    # --- apply for this bc's 16 b ---
                    Y_t = [None] * 8
                    nY = bc
                    for c in range(8):
                        ci = CHUNK_CLS[c]
                        chs, kd, pack, groups = CLASSES[ci]
                        nch = len(chs)
                        cc = c - chs[0]
                        gpb = BC // pack      # groups covering 16 b
                        i0 = bc * gpb
                        psY = pyp.tile([128, 384], f32, tag="psY")
                        for g in range(gpb):
                            i = i0 + g
                            nc.tensor.matmul(
                                psY[:, g * pack * N:(g + 1) * pack * N],
                                L_s[ci][:, (i * nch + cc) * 128:
                                          (i * nch + cc + 1) * 128],
                                T_s[ci][:, i * pack * N:(i + 1) * pack * N],
                                start=True, stop=True)
                        Yc = ypool.tile([128, BC, N], bf16, tag=f"y{c}",
                                        name=f"y{c}_{bc}")
                        cpy(nY, Yc[:],
                            psY[:].rearrange("p (b n) -> p b n", b=BC, n=N))
                        nY += 1
                        Y_t[c] = Yc
                        if DEBUG:
                            nc.scalar.dma_start(
                                dbg_y[c][:, cs:cs + BC, :], Yc[:])

                    # --- c3 ---
                    T3_t = []
                    for m in range(4):
                        ps3 = ps_c3.tile([128, BC, N], f32, tag="c3")
                        for k in range(8):
                            nc.tensor.matmul(
                                ps3[:], c3wt_s[:, k, m * 128:(m + 1) * 128],
                                Y_t[k][:], start=(k == 0), stop=False)
                        nc.tensor.matmul(
                            ps3[:].rearrange("p b n -> p (b n)"),
                            c3bT_s[:, m * 128:(m + 1) * 128], sel24_s[:],
                            start=False, stop=True)
                        T3m = t3p.tile([128, BC, N], bf16, tag="t3")
                        nc.vector.tensor_mul(T3m[:], ps3[:],
                                             bcast(g3_s[:, m, cs:cs + BC]))
                        if DEBUG and bc == 0:
                            nc.scalar.dma_start(dbg_t3[m], T3m[:])
                        T3_t.append(T3m)

                    # --- c4 ---
                    T4_t = []
                    for m in range(2):
                        ps4 = ps_ms.tile([128, BC, N], f32, tag="ms")
                        for k in range(4):
                            nc.tensor.matmul(
                                ps4[:], c4wt_s[:, k, m * 128:(m + 1) * 128],
                                T3_t[k][:], start=(k == 0), stop=False)
                        hf, al = cs // 64, (cs % 64 // 32) * 32
                        selp = selbe_s if bc % 2 == 0 else selbo_s
                        nc.tensor.matmul(
                            ps4[:].rearrange("p b n -> p (b n)"),
                            h4cT[hf][al:al + 32, m, :], selp[al:al + 32, :],
                            start=False, stop=True)
                        T4m = t3p.tile([128, BC, N], bf16, tag="t3")
                        nc.vector.scalar_tensor_tensor(
                            T4m[:], ps4[:], c4b_s[:, m:m + 1],
                            bcast(g4_s[:, m, cs:cs + BC]),
                            AluOp.add, AluOp.mult)
                        if DEBUG and bc == 0:
                            nc.scalar.dma_start(dbg_t4[m], T4m[:])
                        T4_t.append(T4m)

                    # --- cl ---
                    psl_full = ps_ms.tile([128, BC, N], f32, tag="ms")
                    psl = psl_full[0:PD]
                    for k in range(2):
                        nc.tensor.matmul(psl[:], clwt_s[:, k, :], T4_t[k][:],
                                         start=(k == 0), stop=False)
                    hf, al = cs // 64, (cs % 64 // 32) * 32
                    selp = selbe_s if bc % 2 == 0 else selbo_s
                    nc.tensor.matmul(
                        psl[:].rearrange("p b n -> p (b n)"),
                        hlcT[hf][al:al + 32, :], selp[al:al + 32, :],
                        start=False, stop=True)
                    OF_full = t3p.tile([128, BC, N], f32, tag="t3f")
                    OF = OF_full[0:PD]
                    gl = gl_s[:, cs:cs + BC].unsqueeze(2).broadcast_to([PD, BC, N])
                    hl = hl_s[:, cs:cs + BC].unsqueeze(2).broadcast_to([PD, BC, N])
                    if DEBUG and bc == 0:
                        pslc = t3p.tile([128, BC, N], f32, tag="t3f")
                        nc.scalar.copy(pslc[0:PD], psl[:])
                        nc.scalar.dma_start(dbg_psl[:], pslc[0:PD])
                    nc.vector.scalar_tensor_tensor(OF[:], psl[:], clb_s[:], gl,
                                                   AluOp.add, AluOp.mult)
                    nc.vector.tensor_add(OF[:], OF[:], hl)
                    if DEBUG and bc == 0:
                        nc.scalar.dma_start(dbg_of[:], OF[:])

                    # --- transpose [2, 384] -> [384, 2] and DMA out ---
                    OFf = OF[:].rearrange("p b n -> p (b n)")
                    osb = obp.tile([128, 3, PD], f32, tag="ob")
                    for blk in range(3):
                        ptr_full = ps_ms.tile([128, BC, N], f32, tag="ms")
                        ptr = ptr_full.rearrange("p b n -> p (b n)")[:, 0:PD]
                        nc.tensor.transpose(ptr[:],
                                            OFf[:, blk * 128:(blk + 1) * 128],
                                            ident[0:PD, 0:PD])
                        nc.scalar.copy(osb[:, blk, :], ptr[:])
                    row0 = bc * 384
                    oap = out_d[row0:row0 + 384, :].rearrange(
                        "(blk p) c -> p blk c", blk=3, p=128)
                    nc.sync.dma_start(oap, osb[:])

    nc.compile()
    return nc


def _host_prep(**inputs):
    import ml_dtypes
    x = _f32(inputs["x"])
    beta = _f32(inputs["beta"])
    context = _f32(inputs["context"])
    g = {k: np.asarray(v, dtype=np.float64) for k, v in inputs.items()
         if k not in ("x", "beta", "context")}

    # --- phase A folds ---
    embW = g["emb_w"][:, :, 0]
    dembW = g["demb_w"][:, :, 0]
    M3 = dembW @ embW
    v3 = dembW @ g["emb_b"] + g["demb_b"]
    s3 = M3.sum(axis=1)

    # --- conv weights, tap-major, new d-ascending block order ---
    convt = np.zeros((11, F, F), np.float32)   # [d+5, ci, co]
    convt[5, :, 0:512] = g["conv1_w"][:, :, 0].T
    for t in range(3):
        convt[t + 4, :, 512:768] = g["conv2_w"][:, :, t].T
    for t in range(5):
        convt[t + 3, :, 768:832] = g["conv3_w"][:, :, t].T
    for t in range(7):
        convt[t + 2, :, 832:896] = g["conv4_w"][:, :, t].T
    for t in range(9):
        convt[t + 1, :, 896:960] = g["conv5_w"][:, :, t].T
    for t in range(11):
        convt[t, :, 960:1024] = g["conv6_w"][:, :, t].T
    convt_dev = np.empty((8, 128, NBLK * 128), np.float32)
    for (c, d), idx in BLK.items():
        blkslab = convt[d + 5, :, c * 128:(c + 1) * 128]   # [F, 128]
        convt_dev[:, :, idx * 128:(idx + 1) * 128] = blkslab.reshape(8, 128, 128)

    # --- pe + conv-bias pushed through c3 (exact, boundary-aware) ---
    pe = _pe_table().astype(np.float64)
    peT = pe.T
    convt64 = convt.astype(np.float64)
    pe_conv = np.zeros((F, N), np.float64)
    for d in range(-5, 6):
        a, b2 = max(0, -d), N - max(0, d)
        pe_conv[:, a:b2] += convt64[d + 5].T @ peT[:, a + d:b2 + d]
    conv_bias = np.concatenate([g["conv1_b"], g["conv2_b"], g["conv3_b"],
                                g["conv4_b"], g["conv5_b"], g["conv6_b"]])
    c3bias = (g["c3_w"] @ (pe_conv + conv_bias[:, None])
              + g["c3_b"][:, None]).astype(np.float32)   # [C, N]

    # --- gate weight blocks [25, ...] -> wg [128, 100, 128] ---
    W4C = g["c4_w"] @ g["c3_hw"]          # [256, 512]
    WLC = g["cl_w"] @ g["c4_hw"]          # [2, 512]
    wgflat = np.zeros((C, 25 * 128), np.float32)
    wgflat[:, 0:1024] = g["c1_gw"].T
    wgflat[:, 1024:2048] = g["c1_hw"].T
    wgflat[:, 2048:2560] = g["c3_gw"].T
    wgflat[:, 2560:2816] = g["c4_gw"].T
    wgflat[:, 2816:3072] = W4C.T
    wgflat[:, 3072:3074] = g["cl_gw"].T
    wgflat[:, 3074:3076] = g["cl_hw"].T
    wgflat[:, 3076:3078] = WLC.T
    # [q, blk*128+o] -> [p, blk*4+k, o] with q = k*128+p
    wg_dev = np.ascontiguousarray(
        wgflat.reshape(4, 128, 25, 128).transpose(1, 2, 0, 3)
    ).reshape(128, 100, 128)

    gbias = np.zeros(25 * 128, np.float32)
    gbias[0:1024] = g["c1_gb"]
    gbias[2048:2560] = g["c3_gb"]
    gbias[2560:2816] = g["c4_gb"]
    gbias[3072:3074] = g["cl_gb"]

    c1wb = np.stack([g["c1_w"][:, 0], g["c1_w"][:, 1], g["c1_b"]],
                    axis=1).astype(np.float32).reshape(8, 128, 3)
    c1wb = np.ascontiguousarray(c1wb.transpose(1, 0, 2))   # [128, 8, 3]

    c3wt = g["c3_w"].T.reshape(8, 128, C).transpose(1, 0, 2)   # [128, 8, 512]
    c4wt = g["c4_w"].T.reshape(4, 128, 256).transpose(1, 0, 2)
    c4b = g["c4_b"].reshape(2, 128)
    clwt = g["cl_w"].T.reshape(2, 128, PD).transpose(1, 0, 2)
    clb = g["cl_b"].reshape(PD, 1)

    sel24 = np.zeros((N, BC, N), np.float32)
    for n in range(N):
        sel24[n, :, n] = 1.0
    selbe = np.zeros((64, BC, N), np.float32)
    selbo = np.zeros((64, BC, N), np.float32)
    for p in range(64):
        if (p % 32) < 16:
            selbe[p, p % 32, :] = 1.0
        else:
            selbo[p, (p % 32) - 16, :] = 1.0
    idb = np.eye(128, dtype=np.float32)

    host = dict(M3=M3, v3=v3, s3=s3)

    shared = dict(
        wg=_bf16(wg_dev), gbias8=_bf16(gbias.reshape(1, 25 * 128)),
        ones1=_bf16(np.ones((1, 128), np.float32)),
        convt=_bf16(convt_dev),
        c1wb=_f32(c1wb), c3wt=_bf16(c3wt), c3bT=_bf16(c3bias.T),
        sel24=_bf16(sel24.reshape(N, BC * N)),
        selbe=_bf16(selbe.reshape(64, BC * N)),
        selbo=_bf16(selbo.reshape(64, BC * N)),
        c4wt=_bf16(c4wt), c4b=_f32(c4b), clwt=_bf16(clwt), clb=_f32(clb),
        idb=_bf16(idb))

    # --- per-core: ctx, beta, Toeplitz tiles ---
    xpad = np.zeros((B, 3, NW), np.float32)
    xpad[:, 0, PADL:PADL + N] = x[:, :, 0]
    xpad[:, 1, PADL:PADL + N] = x[:, :, 1]
    xpad[:, 2, PADL:PADL + N] = 1.0

    in_maps = []
    for core in range(NCORES):
        sl = slice(core * BLOC, (core + 1) * BLOC)
        xp = xpad[sl]
        m = dict(shared)
        m["ctx"] = np.ascontiguousarray(context[sl])
        m["betav"] = np.ascontiguousarray(beta[sl].reshape(BLOC, 1))
        for ci, (chs, kd, pack, groups) in enumerate(CLASSES):
            kc = 3 * kd
            maxd = (kd - 1) // 2
            tpl = np.zeros((kc * pack, groups, pack, N), np.float32)
            for j in range(pack):
                for r in range(3):
                    for didx in range(kd):
                        d = didx - maxd
                        for i in range(groups):
                            b = i * pack + j
                            tpl[j * kc + r * kd + didx, i, j, :] = \
                                xp[b, r, PADL + d:PADL + d + N]
            m[f"tpl{ci}"] = _bf16(tpl.reshape(kc * pack, groups * pack * N))
        in_maps.append(m)

    return host, in_maps


_LAST_HOST = None


def _build_and_run(host, in_maps, trace):
    from concourse.bass_utils import run_bass_kernel_spmd

    nc = _build(host)
    res = run_bass_kernel_spmd(
        nc, in_maps, core_ids=list(range(NCORES)), trace=trace,
        trace_cores=list(range(NCORES)) if trace else None,
        stitch_traces=bool(trace and NCORES > 1))
    return res


def kernel(**inputs):
    global LAST_RESULTS, _LAST_HOST
    host, in_maps = _host_prep(**inputs)
    _LAST_HOST = host
    trace = bool(int(os.environ.get("KERNEL_TRACE", "0")))
    res = _build_and_run(host, in_maps, trace)
    LAST_RESULTS = res
    out = np.concatenate(
        [res.results[k]["out"].reshape(BLOC, N, PD) for k in range(NCORES)],
        axis=0)
    return out
